# revision 1
# baseline (speedup 1.0000x reference)
"""MCCDecoderAttention Trainium2 kernel (8 NeuronCores).

Sharding: core = b*4 + g  (b in {0,1} batch, g in {0..3} head-group).
Each core computes attention for 3 heads of one batch, plus its partial
contribution to the output projection; the host sums the 4 partials per
batch and adds b_proj.

Device-side layout (per core):
  xT    [768, 2048]  x[b].T (feature-major)
  wqkT  [768, 384]   w_qkv.T columns [q_h0 q_h1 | k_h0 k_h1 | q_h2 | k_h2]
  wvT   [768, 256]   w_qkv.T v-columns [v_h0 v_h1 v_h2 | zeros]
  wpT   [192, 768]   w_proj.T rows for this core's 192 channels
  yT    [768, 2048]  partial output (feature-major)

Attention is computed in S^T orientation (keys on partitions, queries on
the free axis): S^T tiles come straight from matmul(lhsT=K_T, rhs=Q_T),
exp runs on ScalarE (scale=1/8 folded in, no max-subtraction needed for
unit-variance logits), and A@V accumulates over key tiles in PSUM with a
ones-column appended to V so the softmax denominator L falls out of the
same matmuls.  The decoder mask (last `unseen` keys masked except the
diagonal) is handled by only looping keys over [0, N-u) plus an
elementwise diagonal correction for queries in the unseen range.
"""

import functools
import os
import sys

for _p in ("/opt/trn_rl_repo", "/root/.axon_site/_ro/trn_rl_repo"):
    if os.path.isdir(_p) and _p not in sys.path:
        sys.path.insert(0, _p)

import numpy as np

import concourse.bacc as bacc
import concourse.tile as tile
from concourse import mybir

N, C, D = 2048, 768, 64
NH = 3            # heads per core
CT = C // 128     # 6 contraction tiles
NCH = N // 512    # 4 query chunks
F32 = mybir.dt.float32
F32R = mybir.dt.float32r
EXP = mybir.ActivationFunctionType.Exp

_last_results = None  # BassKernelResults of the most recent run (for test.py)


@functools.lru_cache(maxsize=4)
def _build(u: int):
    nc = bacc.Bacc(None, target_bir_lowering=False)
    xT = nc.dram_tensor("xT", [C, N], F32R, kind="ExternalInput")
    wqkT = nc.dram_tensor("wqkT", [C, 6 * D], F32R, kind="ExternalInput")
    wvT = nc.dram_tensor("wvT", [C, 256], F32R, kind="ExternalInput")
    wpT = nc.dram_tensor("wpT", [NH * D, C], F32R, kind="ExternalInput")
    yT = nc.dram_tensor("yT", [C, N], F32, kind="ExternalOutput")

    kfull = N - u
    t_full, rem = divmod(kfull, 128)
    T = t_full + (1 if rem else 0)

    with nc.allow_low_precision(reason="f32r matmul operand staging"), \
         tile.TileContext(nc) as tc:
        with tc.tile_pool(name="persist", bufs=1) as P:
            xt = P.tile([128, CT, N], F32R)
            wqk = P.tile([128, CT, 6 * D], F32R)
            wv = P.tile([128, CT, 256], F32R)
            wp = [P.tile([64, C], F32R, name=f"wp{_h}", tag=f"wp{_h}") for _h in range(NH)]
            qT0 = P.tile([128, N], F32R)   # q heads 0,1 (rows 0-63 / 64-127)
            kT0 = P.tile([128, N], F32R)
            qT1 = P.tile([64, N], F32R)    # q head 2
            kT1 = P.tile([64, N], F32R)
            vsb = P.tile([128, 16, NH, 65], F32R)  # token-major V + ones col
            ao = [P.tile([64, N], F32R, name=f"ao{_h}", tag=f"ao{_h}") for _h in range(NH)]
            ones = P.tile([128, 64], F32R)
            zs = P.tile([128, 1], F32)
            vtu = [P.tile([64, u], F32, name=f"vtu{_h}", tag=f"vtu{_h}") for _h in range(NH)] if u else []

            for ct in range(CT):
                nc.sync.dma_start(xt[:, ct, :], xT[ct * 128:(ct + 1) * 128, :])
            nc.sync.dma_start(wqk[:], wqkT.rearrange("(t p) f -> p t f", p=128))
            nc.sync.dma_start(wv[:], wvT.rearrange("(t p) f -> p t f", p=128))
            for _h in range(NH):
                nc.sync.dma_start(wp[_h][:], wpT[_h * 64:(_h + 1) * 64, :])
            ones_f = P.tile([128, 64], F32)
            nc.vector.memset(ones_f[:], 1.0)
            nc.vector.memset(zs[:], 0.0)
            nc.vector.tensor_copy(ones[:], ones_f[:])
            for h in range(NH):
                nc.vector.tensor_copy(
                    vsb[:, :, h, 64:65],
                    ones_f[:, 0:16].rearrange("p (n x) -> p n x", x=1))

            # ---- phase 1a: Q/K projection (feature-major) ----
            fblocks = [
                (slice(0, 128), 128, qT0, slice(0, 128)),
                (slice(128, 256), 128, kT0, slice(0, 128)),
                (slice(256, 320), 64, qT1, slice(0, 64)),
                (slice(320, 384), 64, kT1, slice(0, 64)),
            ]
            with tc.tile_pool(name="qkv_ps", bufs=4, space="PSUM") as qps:
                for ch in range(NCH):
                    sl = slice(ch * 512, (ch + 1) * 512)
                    for fi, (fc, m, dst, dr) in enumerate(fblocks):
                        ps = qps.tile([m, 512], F32, tag="qkvps", bufs=4)
                        for ct in range(CT):
                            nc.tensor.matmul(
                                ps[:], wqk[:, ct, fc], xt[:, ct, sl],
                                start=(ct == 0), stop=(ct == CT - 1))
                        nc.vector.tensor_copy(dst[dr, sl], ps[:])

                # ---- phase 1b: V projection (token-major, + unseen feature-major) ----
                vps = qps
                for nt in range(16):
                    ps = vps.tile([128, 256], F32, tag="vps", bufs=3)
                    for ct in range(CT):
                        nc.tensor.matmul(
                            ps[:], xt[:, ct, nt * 128:(nt + 1) * 128],
                            wv[:, ct, :],
                            start=(ct == 0), stop=(ct == CT - 1))
                    src = ps[:, 0:192].rearrange("p (h x) -> p h x", x=64)
                    nc.vector.tensor_copy(vsb[:, nt, :, 0:64], src)
                for h in range(NH):
                    for uc in range(0, u, 512):
                        w = min(512, u - uc)
                        ps2 = vps.tile([64, 512], F32, tag="vtu", bufs=1)
                        for ct in range(CT):
                            nc.tensor.matmul(
                                ps2[:, 0:w], wv[:, ct, h * 64:(h + 1) * 64],
                                xt[:, ct, kfull + uc:kfull + uc + w],
                                start=(ct == 0), stop=(ct == CT - 1))
                        nc.vector.tensor_copy(vtu[h][:, uc:uc + w], ps2[:, 0:w])

            # ---- phase 2: attention ----
            heads = [
                (qT0, kT0, 0),
                (qT0, kT0, 64),
                (qT1, kT1, 0),
            ]
            with tc.tile_pool(name="st_ps", bufs=2, space="PSUM") as stps, \
                 tc.tile_pool(name="av_ps", bufs=2, space="PSUM") as avps, \
                 tc.tile_pool(name="bc_ps", bufs=2, space="PSUM") as bcps, \
                 tc.tile_pool(name="a_sb", bufs=4) as apool, \
                 tc.tile_pool(name="scr", bufs=2) as scr:
                for h, (qt, kt, bh) in enumerate(heads):
                    aoh = ao[h]
                    qv = qt[bh:bh + 64, :]
                    kv = kt[bh:bh + 64, :]
                    for p2 in range(2):
                        avs = [avps.tile([65, 512], F32, name=f"av{_i}", tag="av") for _i in range(2)]
                        for t in range(T):
                            st = stps.tile([128, 1024], F32, tag="st")
                            for cc in range(2):
                                qsl = slice(p2 * 1024 + cc * 512,
                                            p2 * 1024 + cc * 512 + 512)
                                nc.tensor.matmul(
                                    st[:, cc * 512:(cc + 1) * 512],
                                    kv[:, t * 128:t * 128 + 128],
                                    qv[:, qsl], start=True, stop=True)
                            a = apool.tile([128, 1024], F32R, tag="a")
                            nc.scalar.activation(a[:], st[:], EXP, scale=0.125)
                            if t == T - 1 and rem:
                                nc.vector.memset(a[rem:128, :], 0.0)
                            for cc in range(2):
                                nc.tensor.matmul(
                                    avs[cc][:], vsb[:, t, h, :],
                                    a[:, cc * 512:(cc + 1) * 512],
                                    start=(t == 0), stop=(t == T - 1),
                                    skip_group_check=True)
                        for cc in range(2):
                            av = avs[cc]
                            qs = p2 * 1024 + cc * 512
                            qe = qs + 512
                            sl = slice(qs, qe)
                            us = max(qs, kfull)
                            # L row (psum partition 64) -> sbuf, then K=1 matmul
                            # broadcasts it to 64 psum partitions.
                            lrow = scr.tile([65, 512], F32R, tag="lrow")
                            nc.vector.tensor_copy(lrow[64:65, :], av[64:65, :])
                            bl = bcps.tile([64, 512], F32, tag="bc")
                            nc.tensor.matmul(bl[:], ones[64:65, 0:64],
                                             lrow[64:65, :], start=True, stop=True)
                            lt = scr.tile([64, 512], F32, tag="lt")
                            rec = scr.tile([64, 512], F32, tag="rec")
                            if us >= qe:  # no masked columns in this chunk
                                nc.vector.tensor_copy(lt[:], bl[:])
                            else:
                                off = us - qs
                                prod = scr.tile([128, 512], F32R, tag="prod")
                                nc.vector.tensor_mul(prod[bh:bh + 64, off:512],
                                                     qt[bh:bh + 64, us:qe],
                                                     kt[bh:bh + 64, us:qe])
                                # M=64 ones matmul: every output row holds the
                                # diagonal score q_i.k_i of column i.
                                dg = bcps.tile([64, 512], F32, tag="bc")
                                nc.tensor.matmul(dg[:, off:512],
                                                 ones[bh:bh + 64, 0:64],
                                                 prod[bh:bh + 64, off:512],
                                                 start=True, stop=True)
                                esb = scr.tile([64, 512], F32, tag="esb")
                                nc.scalar.activation(esb[:, off:512],
                                                     dg[:, off:512],
                                                     EXP, bias=zs[0:64, :],
                                                     scale=0.125)
                                if off:
                                    nc.vector.tensor_copy(lt[:, 0:off],
                                                          bl[:, 0:off])
                                nc.vector.tensor_add(lt[:, off:512],
                                                     bl[:, off:512],
                                                     esb[:, off:512])
                            nc.vector.reciprocal(rec[:], lt[:])
                            nc.vector.tensor_mul(aoh[:, sl], av[0:64, :], rec[:])
                            if us < qe:
                                off = us - qs
                                tmp = scr.tile([64, 512], F32, tag="tmpv")
                                nc.vector.tensor_mul(tmp[:, off:512],
                                                     vtu[h][:, us - kfull:qe - kfull],
                                                     esb[:, off:512])
                                nc.vector.tensor_mul(tmp[:, off:512],
                                                     tmp[:, off:512],
                                                     rec[:, off:512])
                                nc.vector.tensor_add(aoh[:, us:qe],
                                                     aoh[:, us:qe],
                                                     tmp[:, off:512])

            # ---- phase 3: output projection (partial over this core's channels) ----
            with tc.tile_pool(name="pj_ps", bufs=3, space="PSUM") as pjps, \
                 tc.tile_pool(name="ost", bufs=2) as ost:
                for co in range(CT):
                    o = ost.tile([128, N], F32, tag="o")
                    for ch in range(NCH):
                        sl = slice(ch * 512, (ch + 1) * 512)
                        ps = pjps.tile([128, 512], F32, tag="pj")
                        for _h in range(NH):
                            nc.tensor.matmul(ps[:], wp[_h][:, co * 128:(co + 1) * 128],
                                             ao[_h][:, sl],
                                             start=(_h == 0), stop=(_h == NH - 1))
                        nc.vector.tensor_copy(o[:, sl], ps[:])
                    nc.sync.dma_start(yT[co * 128:(co + 1) * 128, :], o[:])

    nc.compile()
    return nc


def kernel(**inputs):
    global _last_results
    from concourse.bass_utils import run_bass_kernel_spmd

    x = np.asarray(inputs["x"], np.float32)
    w_qkv = np.asarray(inputs["w_qkv"], np.float32)
    w_proj = np.asarray(inputs["w_proj"], np.float32)
    b_proj = np.asarray(inputs["b_proj"], np.float32)
    u = int(np.asarray(inputs["unseen_size"]))
    B = x.shape[0]
    H = 12

    nc = _build(u)

    wT = np.ascontiguousarray(w_qkv.T)       # [768, 2304]
    wpT_full = np.ascontiguousarray(w_proj.T)  # [768, 768] (ci, co)
    xTb = [np.ascontiguousarray(x[b].T) for b in range(B)]

    in_maps = []
    for core in range(8):
        b, g = divmod(core, 4)
        hs = [3 * g, 3 * g + 1, 3 * g + 2]
        qcols = [0 * C + h * D + i for h in hs[:2] for i in range(D)]
        kcols = [1 * C + h * D + i for h in hs[:2] for i in range(D)]
        q2 = [0 * C + hs[2] * D + i for i in range(D)]
        k2 = [1 * C + hs[2] * D + i for i in range(D)]
        vcols = [2 * C + h * D + i for h in hs for i in range(D)]
        wqkT = np.ascontiguousarray(wT[:, qcols + kcols + q2 + k2])
        wvT = np.zeros((C, 256), np.float32)
        wvT[:, 0:192] = wT[:, vcols]
        ci = [h * D + i for h in hs for i in range(D)]
        wpT = np.ascontiguousarray(wpT_full[ci, :])
        in_maps.append({"xT": xTb[b], "wqkT": wqkT, "wvT": wvT, "wpT": wpT})

    trace = bool(int(os.environ.get("KERNEL_TRACE", "0")))
    res = run_bass_kernel_spmd(nc, in_maps, core_ids=list(range(8)), trace=trace)
    _last_results = res

    y = np.zeros((B, N, C), np.float32)
    for core in range(8):
        b = core // 4
        y[b] += res.results[core]["yT"].T
    y += b_proj
    return y



# revision 7
# speedup vs baseline: 1.5579x; 1.5579x over previous
"""MCCDecoderAttention Trainium2 kernel (8 NeuronCores).

Sharding: core = b*4 + g  (b in {0,1} batch, g in {0..3} head-group).
Each core computes attention for 3 heads of one batch plus its partial
contribution to the output projection; the host sums the 4 partials per
batch and adds b_proj.

v3 layout (all operands bf16, f32 PSUM accumulation):
  xT    [768, 2048]  x[b].T (feature-major)
  wqkT  [768, 384]   w_qkv.T columns [q_h0 q_h1 | k_h0 k_h1 | q_h2 k_h2]
  wvT   [768, 192]   w_qkv.T v-columns [v_h0 v_h1 v_h2]
  wpT   [192, 768]   w_proj.T rows for this core's 192 channels
  yT    [768, 2048]  partial output (feature-major, f32)

Single fused pipeline: chunked input DMA -> per-chunk Q/K/V projection
-> transposed-S attention (keys on partitions, queries on the free
axis; exp on the Act engine with the 1/8 scale folded in; A@V
accumulates over key tiles with a ones-column appended to V so the
softmax denominator falls out of the same matmuls) -> per-chunk
normalization (reciprocal on DVE, partition-broadcast on the otherwise
idle GpSimd/Pool engine) -> output projection.  Projection and
leftover-QKV matmuls are interleaved into the attention phase as PE
filler so the Tensor engine never idles behind the Act-engine exp
stream.  The decoder mask (last `unseen` keys masked except the
diagonal) is handled by looping keys over [0, N-u) only, plus an
elementwise diagonal correction for queries in the unseen range.
"""

import functools
import os
import sys

for _p in ("/opt/trn_rl_repo", "/root/.axon_site/_ro/trn_rl_repo"):
    if os.path.isdir(_p) and _p not in sys.path:
        sys.path.insert(0, _p)

import numpy as np

import concourse.bacc as bacc
import concourse.tile as tile
from concourse import mybir

N, C, D = 2048, 768, 64
NH = 3            # heads per core
CT = C // 128     # 6 contraction tiles
F32 = mybir.dt.float32
BF = mybir.dt.bfloat16
EXP = mybir.ActivationFunctionType.Exp

_last_results = None  # BassKernelResults of the most recent run (for test.py)


@functools.lru_cache(maxsize=4)
def _build(u: int):
    nc = bacc.Bacc(None, target_bir_lowering=False)
    xT = nc.dram_tensor("xT", [C, N], BF, kind="ExternalInput")
    wqkT = nc.dram_tensor("wqkT", [C, 384], BF, kind="ExternalInput")
    wvT = nc.dram_tensor("wvT", [C, 192], BF, kind="ExternalInput")
    wpT = nc.dram_tensor("wpT", [NH * D, C], BF, kind="ExternalInput")
    yT = nc.dram_tensor("yT", [C, N], F32, kind="ExternalOutput")

    kfull = N - u
    t_full, rem = divmod(kfull, 128)
    T = t_full + (1 if rem else 0)
    fast = (u == 512)  # tuned filler schedule for the benched shape

    with nc.allow_low_precision(reason="bf16 attention staging"), \
         tile.TileContext(nc) as tc, \
         tc.tile_pool(name="persist", bufs=1) as P, \
         tc.tile_pool(name="scr", bufs=2) as S, \
         tc.tile_pool(name="apool", bufs=4) as A, \
         tc.tile_pool(name="opool", bufs=4) as O, \
         tc.tile_pool(name="stp", bufs=2, space="PSUM") as STP, \
         tc.tile_pool(name="avp", bufs=2, space="PSUM") as AVP, \
         tc.tile_pool(name="auxp", bufs=2, space="PSUM") as AUX:
        xt = P.tile([128, CT, N], BF)
        wqk = P.tile([128, CT, 384], BF)
        wv = P.tile([128, CT, 192], BF)
        wp = [P.tile([128, C], BF, name=f"wp{_h}", tag=f"wp{_h}")
              for _h in range(NH)]
        qT0 = P.tile([128, N], BF)
        kT0 = P.tile([128, N], BF)
        qT1 = P.tile([64, N], BF)
        kT1 = P.tile([64, N], BF)
        # per-head 128-wide block: [ones | zeros*63 | V*64] so the A@V
        # matmul lands L on PSUM row 0 and O on rows 64-127 (PE-legal base)
        vsb = P.tile([128, T, NH, 128], BF)
        ao = [P.tile([128, N], BF, name=f"ao{_h}", tag=f"ao{_h}")
              for _h in range(NH)]
        ones = P.tile([128, 64], BF)
        zs = P.tile([128, 1], F32)
        vtu = [P.tile([128, u], F32, name=f"vtu{_h}", tag=f"vtu{_h}")
               for _h in range(NH)] if u else []

        nc.gpsimd.memset(ones[:], 1.0)
        nc.gpsimd.memset(zs[:], 0.0)
        nc.gpsimd.memset(vsb[:], 0.0)
        for h in range(NH):
            nc.gpsimd.memset(vsb[:, :, h, 0:1], 1.0)

        # ---- input DMAs, arrival-ordered for the pipeline ----
        xTr = xT.rearrange("(t p) n -> p t n", p=128)
        wqkr = wqkT.rearrange("(t p) f -> p t f", p=128)
        wvr = wvT.rearrange("(t p) f -> p t f", p=128)
        nc.sync.dma_start(wqk[:, 0:3, :], wqkr[:, 0:3, :])
        nc.sync.dma_start(wqk[:, 3:6, :], wqkr[:, 3:6, :])
        nc.sync.dma_start(xt[:, :, 0:512], xTr[:, :, 0:512])
        nc.sync.dma_start(xt[:, :, 512:1024], xTr[:, :, 512:1024])
        nc.sync.dma_start(wv[:], wvr[:])
        nc.sync.dma_start(xt[:, :, 1024:1536], xTr[:, :, 1024:1536])
        nc.sync.dma_start(xt[:, :, 1536:2048], xTr[:, :, 1536:2048])
        for _h in range(NH):
            nc.sync.dma_start(wp[_h][64:128, :], wpT[_h * 64:(_h + 1) * 64, :])

        # ---- closure factories (each = one PSUM-tile of PE work) ----
        def qkproj(ch, fi):
            def f():
                sl = slice(ch * 512, (ch + 1) * 512)
                ps = AUX.tile([128, 512], F32, tag="aux", name="qkps")
                for ct in range(CT):
                    nc.tensor.matmul(ps[:], wqk[:, ct, fi * 128:(fi + 1) * 128],
                                     xt[:, ct, sl], start=(ct == 0),
                                     stop=(ct == CT - 1), skip_group_check=True)
                if fi == 0:
                    nc.vector.tensor_copy(qT0[:, sl], ps[:])
                elif fi == 1:
                    nc.vector.tensor_copy(kT0[:, sl], ps[:])
                else:
                    nc.vector.tensor_copy(qT1[:, sl], ps[0:64, :])
                    nc.vector.tensor_copy(kT1[:, sl], ps[64:128, :])
            return f

        def vproj(nt):
            def f():
                ps = AUX.tile([128, 192], F32, tag="aux", name="vps")
                for ct in range(CT):
                    nc.tensor.matmul(ps[:], xt[:, ct, nt * 128:(nt + 1) * 128],
                                     wv[:, ct, :], start=(ct == 0),
                                     stop=(ct == CT - 1), skip_group_check=True)
                nc.vector.tensor_copy(
                    vsb[:, nt, :, 64:128],
                    ps[:].rearrange("p (h x) -> p h x", x=64))
            return f

        def vtuproj(h):
            def f():
                for uc in range(0, u, 512):
                    w = min(512, u - uc)
                    ps = AUX.tile([128, 512], F32, tag="aux", name="vtups")
                    for ct in range(CT):
                        nc.tensor.matmul(ps[64:128, 0:w], wv[:, ct, h * 64:(h + 1) * 64],
                                         xt[:, ct, kfull + uc:kfull + uc + w],
                                         start=(ct == 0), stop=(ct == CT - 1),
                                         skip_group_check=True)
                    nc.vector.tensor_copy(vtu[h][64:128, uc:uc + w], ps[64:128, 0:w])
            return f

        def proj(qc, co):
            def f():
                sl = slice(qc * 512, (qc + 1) * 512)
                ps = AUX.tile([128, 512], F32, tag="aux", name="pjps")
                for _h in range(NH):
                    nc.tensor.matmul(ps[:], wp[_h][64:128, co * 128:(co + 1) * 128],
                                     ao[_h][64:128, sl], start=(_h == 0),
                                     stop=(_h == NH - 1),
                                     skip_group_check=True)
                o = O.tile([128, 512], F32, tag="o", name="o")
                nc.vector.tensor_copy(o[:], ps[:])
                nc.sync.dma_start(yT[co * 128:(co + 1) * 128, sl], o[:])
            return f

        # ---- normalization (per 512-query chunk) ----
        # Single-row (L) arithmetic stays on partition 64 to line up with
        # the PSUM L row; GpSimd partition_broadcast fans it back out to
        # partitions 0-63 where the O rows and ao tiles live.
        def normalize(p2, h, cc, av, bh, qv, kv, dst):
            qs = p2 * 1024 + cc * 512
            qe = qs + 512
            sl = slice(qs, qe)
            us = max(qs, kfull)
            if us >= qe:  # no masked columns in this chunk
                rr = S.tile([1, 512], F32, tag="rrow", name="rr")
                nc.vector.reciprocal(rr[0:1, :], av[0:1, :])
                rb = S.tile([128, 512], F32, tag="rbc", name="rb")
                nc.gpsimd.partition_broadcast(rb[:, :], rr[0:1, :])
                nc.vector.tensor_mul(dst[64:128, sl], av[64:128, :],
                                     rb[64:128, :])
            else:
                off = us - qs
                # diagonal scores q_i . k_i for the masked key range
                prod = S.tile([128, 512], BF, tag="prod", bufs=1, name="prod")
                nc.vector.tensor_mul(prod[bh:bh + 64, off:512],
                                     qv[:, us:qe], kv[:, us:qe])
                dg = AUX.tile([64, 512], F32, tag="aux", name="dg")
                nc.tensor.matmul(dg[0:64, off:512], ones[bh:bh + 64, :],
                                 prod[bh:bh + 64, off:512], start=True,
                                 stop=True, skip_group_check=True)
                es = S.tile([1, 512], F32, tag="esb", bufs=1, name="es")
                nc.scalar.activation(es[0:1, off:512], dg[0:1, off:512],
                                     EXP, bias=zs[0:1, :], scale=0.125)
                lr = S.tile([1, 512], F32, tag="lrow", bufs=1, name="lr")
                if off:
                    nc.vector.tensor_copy(lr[0:1, 0:off], av[0:1, 0:off])
                nc.vector.tensor_add(lr[0:1, off:512], av[0:1, off:512],
                                     es[0:1, off:512])
                rr = S.tile([1, 512], F32, tag="rrow", name="rr")
                nc.vector.reciprocal(rr[0:1, :], lr[0:1, :])
                rb = S.tile([128, 512], F32, tag="rbc", name="rb")
                nc.gpsimd.partition_broadcast(rb[:, :], rr[0:1, :])
                nc.vector.tensor_mul(dst[64:128, sl], av[64:128, :],
                                     rb[64:128, :])
                # add back exp(diag)/L * v_i on the diagonal block
                er = S.tile([1, 512], F32, tag="erow", bufs=1, name="er")
                nc.vector.tensor_mul(er[0:1, off:512], es[0:1, off:512],
                                     rr[0:1, off:512])
                eb = S.tile([128, 512], F32, tag="ebc", bufs=1, name="eb")
                nc.gpsimd.partition_broadcast(eb[:, off:512],
                                              er[0:1, off:512])
                tm = S.tile([128, 512], F32, tag="tmpc", bufs=1, name="tm")
                nc.gpsimd.tensor_mul(tm[64:128, off:512],
                                     vtu[h][64:128, us - kfull:qe - kfull],
                                     eb[64:128, off:512])
                nc.vector.tensor_add(dst[64:128, us:qe], dst[64:128, us:qe],
                                     tm[64:128, off:512])

        # ---- attention block: one (query-half, head) pair ----
        def attn_block(p2, h, fillers):
            if h < 2:
                bh = h * 64
                qv = qT0[bh:bh + 64, :]
                kv = kT0[bh:bh + 64, :]
            else:
                bh = 0
                qv = qT1[0:64, :]
                kv = kT1[0:64, :]
            dst = ao[h]
            q0 = p2 * 1024
            avs = [AVP.tile([128, 512], F32, name=f"av{_c}", tag="av")
                   for _c in range(2)]
            pend = {}

            def issue_st(t):
                st = STP.tile([128, 1024], F32, tag="st", name="st")
                for cc in range(2):
                    nc.tensor.matmul(st[:, cc * 512:(cc + 1) * 512],
                                     kv[:, t * 128:t * 128 + 128],
                                     qv[:, q0 + cc * 512:q0 + cc * 512 + 512],
                                     start=True, stop=True,
                                     skip_group_check=True)
                a = A.tile([128, 1024], BF, tag="a", name="a")
                nc.scalar.activation(a[:], st[:], EXP, scale=0.125)
                if t == T - 1 and rem:
                    nc.vector.memset(a[rem:128, :], 0.0)
                pend[t] = a

            issue_st(0)
            if T > 1:
                issue_st(1)
            for t in range(T):
                a = pend.pop(t)
                for cc in range(2):
                    nc.tensor.matmul(avs[cc][:], vsb[:, t, h, :],
                                     a[:, cc * 512:(cc + 1) * 512],
                                     start=(t == 0), stop=(t == T - 1),
                                     skip_group_check=True)
                if t + 2 < T:
                    issue_st(t + 2)
                if t < len(fillers) and fillers[t] is not None:
                    fillers[t]()
            for cc in range(2):
                normalize(p2, h, cc, avs[cc], bh, qv, kv, dst)

        # ---- schedule ----
        if fast:
            # inline: qk ch0, qk ch1, v tiles 0-3 (matches DMA arrival)
            for ch in (0, 1):
                for fi in range(3):
                    qkproj(ch, fi)()
            for nt in range(4):
                vproj(nt)()
            fillers = [
                [vproj(4), vproj(5), qkproj(2, 1), vproj(6), vproj(7),
                 vproj(8), vproj(9), vproj(10), vproj(11), qkproj(2, 2),
                 None, None],
                [qkproj(2, 0), qkproj(3, 0), qkproj(3, 1), qkproj(3, 2),
                 vtuproj(0), vtuproj(1)],
                [vtuproj(2)],
                [proj(0, 0), proj(1, 0), proj(0, 1), proj(1, 1),
                 proj(0, 2), proj(1, 2)],
                [proj(0, 3), proj(1, 3), proj(0, 4), proj(1, 4),
                 proj(0, 5), proj(1, 5)],
                [],
            ]
        else:
            # conservative general-u path: all projections before attention
            for ch in range(4):
                for fi in range(3):
                    qkproj(ch, fi)()
            for nt in range(T):
                vproj(nt)()
            for h in range(NH):
                if u:
                    vtuproj(h)()
            half = [proj(qc, co) for qc in (0, 1) for co in range(CT)]
            fillers = [[], [], [], half[0:4], half[4:8], half[8:12]]

        blocks = [(0, 0), (0, 1), (0, 2), (1, 0), (1, 1), (1, 2)]
        for bi, (p2, h) in enumerate(blocks):
            attn_block(p2, h, fillers[bi])

        # tail: output projection for the second query half
        for co in range(CT):
            proj(2, co)()
        for co in range(CT):
            proj(3, co)()

    nc.compile()
    return nc


def kernel(**inputs):
    global _last_results
    from concourse.bass_utils import run_bass_kernel_spmd
    import ml_dtypes

    BFNP = ml_dtypes.bfloat16
    x = np.asarray(inputs["x"], np.float32)
    w_qkv = np.asarray(inputs["w_qkv"], np.float32)
    w_proj = np.asarray(inputs["w_proj"], np.float32)
    b_proj = np.asarray(inputs["b_proj"], np.float32)
    u = int(np.asarray(inputs["unseen_size"]))
    B = x.shape[0]

    nc = _build(u)

    wT = np.ascontiguousarray(w_qkv.T)         # [768, 2304]
    wpT_full = np.ascontiguousarray(w_proj.T)  # [768, 768] (ci, co)
    xTb = [np.ascontiguousarray(x[b].T).astype(BFNP) for b in range(B)]

    in_maps = []
    for core in range(8):
        b, g = divmod(core, 4)
        hs = [3 * g, 3 * g + 1, 3 * g + 2]
        qcols = [0 * C + h * D + i for h in hs[:2] for i in range(D)]
        kcols = [1 * C + h * D + i for h in hs[:2] for i in range(D)]
        q2 = [0 * C + hs[2] * D + i for i in range(D)]
        k2 = [1 * C + hs[2] * D + i for i in range(D)]
        vcols = [2 * C + h * D + i for h in hs for i in range(D)]
        wqkTc = np.ascontiguousarray(
            wT[:, qcols + kcols + q2 + k2]).astype(BFNP)
        wvTc = np.ascontiguousarray(wT[:, vcols]).astype(BFNP)
        ci = [h * D + i for h in hs for i in range(D)]
        wpTc = np.ascontiguousarray(wpT_full[ci, :]).astype(BFNP)
        in_maps.append({"xT": xTb[b], "wqkT": wqkTc, "wvT": wvTc,
                        "wpT": wpTc})

    trace = bool(int(os.environ.get("KERNEL_TRACE", "0")))
    res = run_bass_kernel_spmd(nc, in_maps, core_ids=list(range(8)),
                               trace=trace)
    _last_results = res

    y = np.zeros((B, N, C), np.float32)
    for core in range(8):
        b = core // 4
        y[b] += res.results[core]["yT"].T
    y += b_proj
    return y
